# revision 100
# baseline (speedup 1.0000x reference)
"""AllAttention kernel for Trainium2 (8 NeuronCores, pure data parallel).

Computation (per batch item b):
    att   = feats[b] @ Wf + bf            # [A, H]
    att_h = h[b] @ Wh + bh                # [H]
    dot   = tanh(att + att_h)             # [A, H]
    s     = dot @ wa (+ ba)               # [A]   (ba dropped: softmax shift-invariant)
    w     = softmax(s)                    # [A]
    out   = w @ feats[b]                  # [R]

Shapes: B=256, A=196, R=1024, H=512. Sharded: batch/8 per core (32 each).

Per-core design (pairs of batch items flow through a software pipeline):
  load   : fp32 feats in 4-batch blocks via SWDGE (Pool ring), a1 rows
           before a0 rows (the a1 PE-transposes are the first consumer)
  xform  : DVE cast fp32->bf16; a0 rows (128) transposed by one batched
           SBUF->SBUF DMA-xbar transpose (SP ring); a1 rows (68) transposed
           on the PE (16 small transpose matmuls -> PSUM -> ACT copy), so
           no pad memsets and no second xbar transpose.  The DVE queue
           carries only load-driven casts so it never couples the load
           pipeline to the compute clock.
  mm1    : att^T = Wf^T @ feats^T, reading the xbar tile (a0) and the
           PE-transposed tile (a1) directly as two accumulation regions
           per (h-chunk); tanh+bias fused on ACT (bias = per-partition
           beta^T column, beta precomputed on PE from Wh/h/bf/bh)
  scores : PE matmul with wa stationary; exp+row-sum fused on ACT
           (accum_out -> per-group sums row); UNNORMALIZED exp columns
           made by K=1 PE matmuls (er_chunk^T @ [1])
  mm2    : out^T columns via N=1 matmuls with feats-natural as the
           stationary operand (Ldweights is cheap; avoids the N=512 M=32
           waste of a row-major mm2).  Results accumulate as out^T[r, b]
           in PSUM; each group of 4 batches is drained by one ACT copy,
           transposed back to [b, r] rows by one PE transpose, scaled by
           the per-batch 1/sum(exp) (softmax normalization applied at the
           very end, where batch sits on partitions), and stored with a
           single contiguous DMA.
The softmax/mm2 tail of pair p is spread over the mm1 chunks of pairs
p+1 and p+2 so the in-order PE queue never head-blocks on the
cross-engine softmax chain.  Weight loads go over the ACT DMA ring,
feats loads over the Pool ring, and xbar transposes + stores over the
SP ring so no ring head-blocks another.
"""

from contextlib import ExitStack

import numpy as np

import concourse.bacc as bacc
import concourse.mybir as mybir
import concourse.tile as tile
from concourse.bass import ds, ts
from concourse.bass_utils import run_bass_kernel_spmd
from concourse.masks import make_identity

F32 = mybir.dt.float32
BF16 = mybir.dt.bfloat16
TANH = mybir.ActivationFunctionType.Tanh
EXP = mybir.ActivationFunctionType.Exp

B, A, R, H = 256, 196, 1024, 512
N_CORES = 8
BL = B // N_CORES          # 32 batch items per core
A0 = 128                   # first a-chunk (DMA-xbar transposed)
A1 = A - A0                # 68  (PE transposed)
RC = R // 128              # 8 r-chunks
HC = H // 128              # 4 h-chunks
NPAIRS = BL // 2
NGROUPS = BL // 4


def _emit(tc):
    nc = tc.nc
    ctx = ExitStack()

    h_d = nc.dram_tensor("h_in", [BL, R], F32, kind="ExternalInput").ap()
    feats_d = nc.dram_tensor("feats_in", [BL, A, R], F32, kind="ExternalInput").ap()
    wf_d = nc.dram_tensor("wf_in", [R, H], F32, kind="ExternalInput").ap()
    bf_d = nc.dram_tensor("bf_in", [H], F32, kind="ExternalInput").ap()
    wh_d = nc.dram_tensor("wh_in", [R, H], F32, kind="ExternalInput").ap()
    bh_d = nc.dram_tensor("bh_in", [H], F32, kind="ExternalInput").ap()
    wa_d = nc.dram_tensor("wa_in", [H], F32, kind="ExternalInput").ap()
    out_d = nc.dram_tensor("out", [BL, R], F32, kind="ExternalOutput").ap()

    singles = ctx.enter_context(tc.tile_pool(name="singles", bufs=1))

    ident = singles.tile([128, 128], F32)
    make_identity(nc, ident)
    ident_bf = singles.tile([128, 128], BF16)
    nc.vector.tensor_copy(out=ident_bf, in_=ident)

    wa_sb = singles.tile([128, HC], BF16)       # wa[128*c + p] -> [p, c]
    betaT = singles.tile([128, HC, BL], F32)    # beta^T[h, b] per h-chunk
    wf_bf = singles.tile([128, RC, H], BF16)    # Wf bf16, r on partitions
    # out^T staging: outT_sb[:, g, jb, rc] = out[4g+jb][rc*128 + p]
    outT_sb = singles.tile([128, NGROUPS, 4, RC], F32)
    ones11 = singles.tile([1, 1], F32)
    nc.vector.memset(ones11, 1.0)
    # REP[j, j*8+i] = 1: replicates a [4,1] per-batch column to [32,1]
    # (constructed after the prologue loads so its gpsimd ops don't delay
    # the first feats descriptor generation on the Pool engine)
    rep4 = singles.tile([4, 32], F32)

    def build_rep4():
        nc.gpsimd.memset(rep4, 1.0)
        nc.gpsimd.affine_select(
            out=rep4, in_=rep4, compare_op=mybir.AluOpType.is_ge, fill=0.0,
            base=0, pattern=[[1, 32]], channel_multiplier=-8,
        )
        nc.gpsimd.affine_select(
            out=rep4, in_=rep4, compare_op=mybir.AluOpType.is_ge, fill=0.0,
            base=7, pattern=[[-1, 32]], channel_multiplier=8,
        )

    # ---- main pools ----
    fnat = ctx.enter_context(tc.tile_pool(name="fnat", bufs=3))
    fbf = ctx.enter_context(tc.tile_pool(name="fbf", bufs=5))
    ftp = ctx.enter_context(tc.tile_pool(name="ftp", bufs=5))
    ta1p = ctx.enter_context(tc.tile_pool(name="ta1p", bufs=2))
    dtp = ctx.enter_context(tc.tile_pool(name="dtp", bufs=3))
    erow = ctx.enter_context(tc.tile_pool(name="erow", bufs=2))
    ecol = ctx.enter_context(tc.tile_pool(name="ecol", bufs=6))
    rsb = ctx.enter_context(tc.tile_pool(name="rsb", bufs=2))
    stage = ctx.enter_context(tc.tile_pool(name="stage", bufs=2))
    setup_sb = ctx.enter_context(tc.tile_pool(name="setup_sb", bufs=1))

    mp_ps = ctx.enter_context(tc.tile_pool(name="mp_ps", bufs=2, space="PSUM"))
    sc_ps = ctx.enter_context(tc.tile_pool(name="sc_ps", bufs=2, space="PSUM"))
    oT_ps = ctx.enter_context(tc.tile_pool(name="oT_ps", bufs=2, space="PSUM"))
    tq_ps = ctx.enter_context(tc.tile_pool(name="tq_ps", bufs=2, space="PSUM"))

    # ---- pipeline state ----
    fblks = {}   # block k -> (fc0, fc1) fp32 natural, pairs 2k and 2k+1
    fbigs = {}   # pair -> (fbig0, fbig1) bf16 natural (kept until mm2)
    tps = {}     # pair -> tp (a0 transposed, [128, 2*RC, 128])
    ta1s = {}    # pair -> ta1 (a1 transposed, [128, 2, RC, A1])
    dts = {}     # pair -> dt_t tanh output
    ecs_all = {} # pair -> {"er":..., s: ec}
    scbs = {}    # pair -> scores psum tile
    group_oT = {}

    def stage_load(k, split=False):
        # one 4-batch block per DMA pair: few SWDGE dispatches, slow
        # rotation of the 8 SW DMA-completion semaphores.  split=True uses
        # pair-sized halves so the first casts can start sooner (startup).
        b0 = 4 * k
        fc0 = fnat.tile([128, 4, R], F32, tag="fc0", name="fc0")
        fc1 = fnat.tile([A1, 4, R], F32, tag="fc1", name="fc1")
        halves = ((0, 2), (2, 4)) if split else ((0, 4),)
        for lo, hi in halves:
            # a1 rows first: the PE a1-transposes are the earliest consumer
            nc.gpsimd.dma_start(
                out=fc1[:, lo:hi, :],
                in_=feats_d[b0 + lo : b0 + hi, A0:A, :].rearrange("s p r -> p s r"),
            )
            nc.gpsimd.dma_start(
                out=fc0[:, lo:hi, :],
                in_=feats_d[b0 + lo : b0 + hi, 0:A0, :].rearrange("s p r -> p s r"),
            )
        fblks[k] = (fc0, fc1)

    def stage_xform(q):
        """Cast fp32->bf16 (DVE) and launch the a0 xbar transpose (SP ring).

        tp[:, s*RC+rc, :] = feats[2q+s][0:128, ts(rc,128)].T  (bf16)
        """
        fc0, fc1 = fblks[q // 2]
        j = 2 * (q % 2)
        fbig0 = fbf.tile([128, 2, R], BF16, tag="fb0", name="fb0")
        fbig1 = fbf.tile([A1, 2, R], BF16, tag="fb1", name="fb1")
        nc.vector.tensor_copy(out=fbig1, in_=fc1[:, j : j + 2, :])
        nc.vector.tensor_copy(out=fbig0, in_=fc0[:, j : j + 2, :])
        if q % 2 == 1:
            del fblks[q // 2]
        tp = ftp.tile([128, 2 * RC, A0], BF16, tag="tp", name="tp")
        nc.sync.dma_start(
            out=tp, in_=fbig0.rearrange("p s r -> p (s r)"), transpose=True
        )
        fbigs[q] = (fbig0, fbig1)
        tps[q] = tp

    def stage_a1t(q):
        """PE-transpose the 68-row a1 blocks into PSUM, copy to SBUF."""
        fbig1 = fbigs[q][1]
        ta1 = ta1p.tile([128, 2, RC, A1], BF16, tag="ta1", name="ta1")
        for s in range(2):
            tq = tq_ps.tile([128, RC, 72], BF16, tag="tq", name="tq")
            for rc in range(RC):
                nc.tensor.transpose(
                    tq[:, rc, 0:A1],
                    fbig1[:, s, ts(rc, 128)],
                    ident_bf[0:A1, 0:A1],
                )
            # ACT does the PSUM->SBUF merge so the DVE queue stays a pure
            # load->cast chain (no coupling to the PE/softmax clock)
            nc.scalar.copy(out=ta1[:, s], in_=tq[:, :, 0:A1])
        ta1s[q] = ta1

    # ---- setup: h / biases / wa; weight loads per h-column-block ----
    # h shares the weight-staging rotation (first allocation -> first buf)
    h_sb = setup_sb.tile([BL, R], F32, tag="ws", bufs=3, name="h_sb")
    nc.sync.dma_start(out=h_sb, in_=h_d)
    bf_sb = setup_sb.tile([1, H], F32, name="bf_sb")
    bh_sb = setup_sb.tile([1, H], F32, name="bh_sb")
    nc.sync.dma_start(out=bf_sb, in_=bf_d[None, :])
    nc.sync.dma_start(out=bh_sb, in_=bh_d[None, :])
    wa_f = setup_sb.tile([128, HC], F32, name="wa_f")
    nc.sync.dma_start(out=wa_f, in_=wa_d.rearrange("(c p) -> p c", p=128))

    # feats loads first on the Pool ring so they lead DMA arbitration;
    # xbar transposes claim their HWDGE semaphores before the weight loads
    # (otherwise tp0 serializes behind the weight/beta chain)
    stage_load(0, split=True)
    stage_xform(0)
    stage_load(1)
    stage_xform(1)
    stage_load(2)
    build_rep4()

    # weight column-blocks on the ACT ring: wf[hc] before wh[hc]; per-hc
    # availability lets mm1/tanh start before the full weights arrive.
    wcast = {}
    for hc in range(HC):
        for wd, nm in ((wf_d, "wf"), (wh_d, "wh")):
            wtmp = setup_sb.tile(
                [128, RC, 128], F32, tag="ws", bufs=3, name="wtmp"
            )
            nc.scalar.dma_start(
                out=wtmp, in_=wd[:, ts(hc, 128)].rearrange("(rc p) h -> p rc h", p=128)
            )
            wcast[(nm, hc)] = wtmp

    def cast_weights(hc):
        nc.vector.tensor_copy(
            out=wf_bf[:, :, ts(hc, 128)], in_=wcast.pop(("wf", hc))
        )

    nc.vector.tensor_copy(out=wa_sb, in_=wa_f)
    bfh = setup_sb.tile([1, H], BF16, name="bfh")
    nc.vector.tensor_add(out=bfh, in0=bf_sb, in1=bh_sb)
    ones_row = setup_sb.tile([1, BL], BF16, name="ones_row")
    nc.vector.memset(ones_row, 1.0)

    # h [BL, R] -> hT [128, rc, BL] via PE transposes (first PE work)
    hT_pt = sc_ps.tile([128, 512], F32, tag="sc", name="hT_pt")
    hT_ps = hT_pt[:, 0 : RC * BL]
    for rc in range(RC):
        nc.tensor.transpose(
            hT_ps[:, ts(rc, BL)], h_sb[:, ts(rc, 128)], ident[0:BL, 0:BL]
        )
    hT = setup_sb.tile([128, RC, BL], BF16, name="hT")
    nc.vector.tensor_copy(out=hT, in_=hT_ps.rearrange("p (rc b) -> p rc b", rc=RC))

    def beta_mm(hc, with_cast=True):
        # betaT[:, hc, :] = Wh[:, hc-chunk]^T @ h^T + (bf+bh)[hc-chunk]
        if with_cast:
            cast_weights(hc)
        whb = setup_sb.tile([128, RC, 128], BF16, tag="whb", bufs=2, name="whb")
        nc.vector.tensor_copy(out=whb, in_=wcast.pop(("wh", hc)))
        bps_t = sc_ps.tile([128, 512], F32, tag="sc", name="bps_t")
        bps = bps_t[:, 0:BL]
        for rc in range(RC):
            nc.tensor.matmul(
                bps,
                lhsT=whb[:, rc, :],
                rhs=hT[:, rc, :],
                start=(rc == 0),
                stop=False,
            )
        nc.tensor.matmul(
            bps,
            lhsT=bfh[0:1, ts(hc, 128)],
            rhs=ones_row,
            start=False,
            stop=True,
        )
        nc.vector.tensor_copy(out=betaT[:, hc, :], in_=bps)

    # ---- per-pair pieces ----

    def mm1_tanh(pp, slots, defer_tanh=()):
        """Pair pp's mm1+tanh; slots[hc] closures run before each h-chunk.
        h-chunks in defer_tanh get their tanh (and a just-in-time beta) after
        all matmuls -- used by pair 0 so the wh loads leave the startup DMA
        critical path."""
        tp = tps.pop(pp)
        ta1 = ta1s.pop(pp)
        dt_t = dtp.tile([128, HC, 2, A], BF16, tag="dt_t", name="dt_t")
        deferred = []

        def tanh_hc(hc, mp):
            for s in range(2):
                nc.scalar.activation(
                    out=dt_t[:, hc, s, :],
                    in_=mp[:, :, s],
                    func=TANH,
                    bias=betaT[:, hc, 2 * pp + s : 2 * pp + s + 1],
                    scale=1.0,
                )

        for hc in range(HC):
            for piece in slots[hc]:
                piece()
            # mp is a-major [128, A, 2] so the a0/a1 regions are contiguous
            mp = mp_ps.tile([128, A, 2], F32, tag="mp", name="mp")
            for rc in range(RC):
                nc.tensor.matmul(
                    mp[:, A0:A, :],
                    lhsT=wf_bf[:, rc, ts(hc, 128)],
                    rhs=ta1[:, :, rc, :].rearrange("p s a -> p a s"),
                    start=(rc == 0),
                    stop=(rc == RC - 1),
                )
            for rc in range(RC):
                nc.tensor.matmul(
                    mp[:, 0:A0, :],
                    lhsT=wf_bf[:, rc, ts(hc, 128)],
                    rhs=tp[:, rc : rc + RC + 1 : RC, :].rearrange("p s a -> p a s"),
                    start=(rc == 0),
                    stop=(rc == RC - 1),
                )
            if hc in defer_tanh:
                deferred.append((hc, mp))
            else:
                tanh_hc(hc, mp)
        for hc, mp in deferred:
            beta_mm(hc, with_cast=False)
            tanh_hc(hc, mp)
        dts[pp] = dt_t

    grows = {}   # group -> [1, 4] row of exp-sums (one per batch)

    def p_scores(pp):
        dt_t = dts.pop(pp)
        g = pp // 2
        if pp % 2 == 0:
            grows[g] = rsb.tile([1, 4], F32, tag="grow", name="grow")
        grow = grows[g]
        scb = sc_ps.tile([128, 512], F32, tag="sc", name="scb")
        sc = scb[0:1, 0 : 2 * A]
        for hc in range(HC):
            nc.tensor.matmul(
                sc,
                lhsT=wa_sb[:, hc : hc + 1],
                rhs=dt_t[:, hc, :, :],
                start=(hc == 0),
                stop=(hc == HC - 1),
            )
        er = erow.tile([1, 2 * A], F32, tag="er", name="er")
        j0 = 2 * (pp % 2)
        for s in range(2):
            nc.scalar.activation(
                out=er[0:1, ts(s, A)], in_=sc[0:1, ds(s * A, A)], func=EXP,
                accum_out=grow[0:1, j0 + s : j0 + s + 1],
            )
        scbs[pp] = scb
        ecs_all[pp] = {"er": er}

    def p_expt(pp):
        # UNNORMALIZED exp columns via K=1 PE transpose-matmuls; the 1/sum
        # scaling happens per group in p_final where batch sits on partitions
        ecs = ecs_all[pp]
        er = ecs["er"]
        scb = scbs.pop(pp)
        for s in range(2):
            et = scb[:, 400 + 2 * s : 402 + 2 * s]
            nc.tensor.matmul(
                et[:, 0:1],
                lhsT=er[0:1, ds(s * A, A0)],
                rhs=ones11,
                start=True,
                stop=True,
            )
            nc.tensor.matmul(
                et[0:A1, 1:2],
                lhsT=er[0:1, ds(s * A + A0, A1)],
                rhs=ones11,
                start=True,
                stop=True,
            )
            ec = ecol.tile([128, 2], BF16, tag="ec", name="ec")
            nc.scalar.copy(out=ec[:, 0:1], in_=et[:, 0:1])
            nc.scalar.copy(out=ec[0:A1, 1:2], in_=et[0:A1, 1:2])
            ecs[s] = ec

    def p_mm2(pp):
        # out^T[:, jb, rc] += feats[b]^T @ w[b]; feats natural is the
        # stationary operand, the softmax column the N=1 moving rhs.
        g = pp // 2
        if pp % 2 == 0:
            group_oT[g] = oT_ps.tile([128, 4, RC], F32, tag="oT", name="oT")
        oT = group_oT[g]
        fbig0, fbig1 = fbigs.pop(pp)
        ecs = ecs_all.pop(pp)
        for s in range(2):
            jb = (2 * pp + s) % 4
            ec = ecs[s]
            for rc in range(RC):
                nc.tensor.matmul(
                    oT[:, jb, rc : rc + 1],
                    lhsT=fbig0[:, s, ts(rc, 128)],
                    rhs=ec[:, 0:1],
                    start=True,
                    stop=False,
                )
                nc.tensor.matmul(
                    oT[:, jb, rc : rc + 1],
                    lhsT=fbig1[:, s, ts(rc, 128)],
                    rhs=ec[0:A1, 1:2],
                    start=False,
                    stop=True,
                )

    def p_drain(pp):
        g = pp // 2
        nc.scalar.copy(out=outT_sb[:, g], in_=group_oT.pop(g))

    sts = {}

    c32s = {}

    def p_final_prep(g):
        # per-batch 1/sum(exp) replicated to a [32,1] partition column
        grow = grows.pop(g)
        growr = rsb.tile([1, 4], F32, tag="growr", name="growr")
        nc.vector.reciprocal(out=growr, in_=grow)
        T = sc_ps.tile([32, 512], F32, tag="sc", name="T")
        nc.tensor.matmul(
            T[0:4, 300:301], lhsT=growr, rhs=ones11, start=True, stop=True
        )
        c4sb = stage.tile([4, 1], F32, tag="c4", name="c4sb")
        nc.scalar.copy(out=c4sb, in_=T[0:4, 300:301])
        nc.tensor.matmul(
            T[0:32, 302:303], lhsT=rep4, rhs=c4sb, start=True, stop=True
        )
        c32sb = stage.tile([32, 1], F32, tag="c32", name="c32sb")
        nc.scalar.copy(out=c32sb, in_=T[0:32, 302:303])
        c32s[g] = (T, c32sb)

    def p_final_tr(g):
        # transpose out^T group back to [b, r] rows and normalize
        T, c32sb = c32s.pop(g)
        nc.tensor.transpose(
            T[:, 0:128], outT_sb[:, g].rearrange("p a b -> p (a b)"), ident
        )
        st = stage.tile([32, 128], F32, tag="st", name="st")
        nc.vector.tensor_scalar_mul(st, T[:, 0:128], c32sb)
        sts[g] = st

    def p_final(g):
        p_final_prep(g)
        p_final_tr(g)

    def p_store(g):
        # issued a pair after p_final so it never head-blocks the SP ring
        nc.sync.dma_start(
            out=out_d[ts(g, 4), :].rearrange("b (rc r) -> (b rc) r", r=128),
            in_=sts.pop(g),
        )

    # ---- prologue: fill the pipeline ----
    stage_a1t(0)

    # ---- main loop ----
    for pp in range(NPAIRS):
        slots = [[], [], [], []]
        if pp == 0:
            slots[0].append(lambda: beta_mm(0))
            slots[1].append(lambda: beta_mm(1))
            slots[2].append(lambda: beta_mm(2))
            slots[2].append(lambda: beta_mm(3))
        if pp % 2 == 0 and (pp + 6) // 2 < NPAIRS // 2:
            slots[0].append(lambda k=(pp + 6) // 2: stage_load(k))
        if pp >= 2:
            slots[0].append(lambda q=pp - 2: p_mm2(q))
            if (pp - 2) % 2 == 1:
                slots[1].append(lambda q=pp - 2: p_drain(q))
        if pp + 1 < NPAIRS:
            slots[1].append(lambda q=pp + 1: stage_a1t(q))
        if pp >= 1:
            slots[1].append(lambda q=pp - 1: p_scores(q))
        if pp + 2 < NPAIRS:
            slots[2].append(lambda q=pp + 2: stage_xform(q))
        if pp >= 1:
            slots[3].append(lambda q=pp - 1: p_expt(q))
        if pp >= 4 and pp % 2 == 0:
            slots[3].append(lambda g=(pp - 4) // 2: p_final(g))
        if pp >= 5 and pp % 2 == 1:
            slots[1].append(lambda g=(pp - 5) // 2: p_store(g))
        mm1_tanh(pp, slots)

    # ---- epilogue ----
    p_scores(NPAIRS - 1)
    p_final(NGROUPS - 2)
    p_expt(NPAIRS - 1)
    p_store(NGROUPS - 2)
    p_mm2(NPAIRS - 2)
    p_final_prep(NGROUPS - 1)
    p_mm2(NPAIRS - 1)
    p_drain(NPAIRS - 1)
    p_final_tr(NGROUPS - 1)
    p_store(NGROUPS - 1)
    ctx.close()


_CACHE = {}


def _build():
    if "nc" in _CACHE:
        return _CACHE["nc"]
    nc = bacc.Bacc(
        "TRN2",
        target_bir_lowering=False,
        debug=False,
        enable_asserts=False,
        num_devices=N_CORES,
        dynamic_dma_scratch_size=16384,
        
    )
    with tile.TileContext(nc) as tc:
        _emit(tc)
    nc.compile()
    _CACHE["nc"] = nc
    return nc


def kernel(h, feats, Wf, bf, Wh, bh, wa, ba=None, **_unused):
    h = np.ascontiguousarray(np.asarray(h, dtype=np.float32))
    feats = np.ascontiguousarray(np.asarray(feats, dtype=np.float32))
    Wf = np.ascontiguousarray(np.asarray(Wf, dtype=np.float32))
    bf = np.ascontiguousarray(np.asarray(bf, dtype=np.float32))
    Wh = np.ascontiguousarray(np.asarray(Wh, dtype=np.float32))
    bh = np.ascontiguousarray(np.asarray(bh, dtype=np.float32))
    wa = np.ascontiguousarray(np.asarray(wa, dtype=np.float32))

    nc = _build()
    in_maps = []
    for i in range(N_CORES):
        sl = slice(i * BL, (i + 1) * BL)
        in_maps.append(
            {
                "h_in": np.ascontiguousarray(h[sl]),
                "feats_in": np.ascontiguousarray(feats[sl]),
                "wf_in": Wf,
                "bf_in": bf,
                "wh_in": Wh,
                "bh_in": bh,
                "wa_in": wa,
            }
        )
    res = run_bass_kernel_spmd(nc, in_maps, core_ids=list(range(N_CORES)))
    out = np.concatenate([res.results[i]["out"] for i in range(N_CORES)], axis=0)
    return out.astype(np.float32)


if __name__ == "__main__":
    rng = np.random.default_rng(0)
    s_f = 1.0 / np.sqrt(R)
    s_a = 1.0 / np.sqrt(H)
    inputs = {
        "h": rng.standard_normal((B, R), dtype=np.float32),
        "feats": rng.standard_normal((B, A, R), dtype=np.float32),
        "Wf": rng.uniform(-s_f, s_f, (R, H)).astype(np.float32),
        "bf": rng.uniform(-s_f, s_f, (H,)).astype(np.float32),
        "Wh": rng.uniform(-s_f, s_f, (R, H)).astype(np.float32),
        "bh": rng.uniform(-s_f, s_f, (H,)).astype(np.float32),
        "wa": rng.uniform(-s_a, s_a, (H,)).astype(np.float32),
        "ba": np.float32(0.1),
    }
    out = kernel(**inputs)
    print(out.shape, out.dtype, np.abs(out).mean())
